# revision 63
# baseline (speedup 1.0000x reference)
"""Trainium2 Bass kernel for nn_AttentionHead (B=4, N=2048, d_model=1024, d_k=64).

Sharding: data-parallel over (batch, query-half) -> 8 cores. Each core gets
qT[b, :, h*1024:(h+1)*1024], full kT[b], vT[b] (host pre-transposes so d_model
lands on SBUF partitions), plus packed projection weights. Matmuls are bf16
with fp32 PSUM accumulation.

Per-core device program (the ACT exp stream, 16x1038ns, is the critical
resource; everything is scheduled around starting it early and never
starving it):
  1. Chunk-major DMA granules of qT/kT stream on the SP/Pool/ACT queues in
     arrival-priority order (q first - scores need all of q_^T but only one
     key tile). ACT carries only two small granules, then preloads the exp
     table so the exp stream starts the moment scores tile 0 lands.
  2. q_^T[64,1024] and k chunk 0 project transposed, accumulating per
     512-chunk in PSUM; DVE writebacks fold the biases. k tiles 0-3 borrow
     the first scores buffer (idle until sc1). Only q and k-tile-0 gate the
     loop. k chunks 1-3 project non-transposed in the loop's shadow (64-col
     matmuls, 2.5x cheaper) + 4 PE transposes each, rotating through one
     PSUM bank; the v chain rotates through another.
  3. Main loop over 16 key tiles: scores^T tile [128,1024] in PSUM (2x2
     banks, contraction over just the 64 live partitions), one Exp ACTIVATE
     per tile (scale=1/sqrt(dk) folded) into a persistent e buffer. v chunks
     project non-transposed straight into v_aug[keys,16,65], col 64 = ones.
  4. Out-matmuls are query-major: oacc[qi][128q, 65] += e_t[:,qi]^T @ v_aug_t
     - only a 65-col moving dim on the PE; 8 accumulation groups packed into
     two PSUM banks with one start/stop per bank (zero-region semantics).
     Row 64 accumulates the softmax denominator via the ones column. The
     final tile runs qi descending so oacc1 closes first for the tail.
  5. No on-device normalize: oacc writes back on DVE and ACT in parallel and
     DMAs out partition-major (one 1KB descriptor per partition per half);
     the host divides by column 64 and adds the v-bias (attn rows sum to 1).

A small legalization pass hoists excess per-instruction semaphore waits onto
same-engine NoOps (this container's walrus accepts at most one).
"""

import numpy as np
import ml_dtypes

import concourse.bass as bass
import concourse.tile as tile
from concourse import mybir
from concourse.bass_utils import run_bass_kernel_spmd

B, N, DM, DK = 4, 2048, 1024, 64
NCORES = 8
NQ = N // 2          # queries per core
NK = N               # keys per core
P = 128
NDM = DM // P        # 8 d_model tiles
NKT = NK // P        # 16 key tiles
NQT = NQ // P        # 8 query tiles
DT = mybir.dt.bfloat16
F32 = mybir.dt.float32
BF = ml_dtypes.bfloat16

ODELAY = 4           # out-matmul lag (tiles) behind the exp stream


# --- walrus wait legalization -------------------------------------------------
# The walrus build in this container accepts at most 1 sync wait + 1 sync
# update per instruction (2 for EventSemaphore). Excess WAITS are hoisted
# onto same-engine NoOps placed just before (queues issue in order, so the
# gating is preserved). Updates are completion-signals and stay put.

def _caps(inst):
    opcode = type(inst).__name__
    if opcode == "InstEventSemaphore":
        return 2, 2
    return 1, 1


def _legalize_waits(nc):
    for f in nc.m.functions:
        for bb in f.blocks:
            out = []
            changed = False
            for inst in bb.instructions:
                si = inst.sync_info
                waits = list(si.on_wait) if si is not None else []
                updates = list(si.on_update) if si is not None else []
                wcap, ucap = _caps(inst)
                if len(waits) <= wcap and len(updates) <= ucap:
                    out.append(inst)
                    continue
                changed = True
                keep_w = waits[len(waits) - wcap:] if wcap else []
                extra_w = waits[: len(waits) - wcap] if wcap else waits
                # Updates signal instruction COMPLETION (writes landed);
                # a following NoOp fires at issue time instead, which races
                # consumers against in-flight writes. Never hoist them.
                assert len(updates) <= ucap, (
                    f"{inst.name}: {len(updates)} sync updates exceed the "
                    f"per-instruction cap and cannot be hoisted safely"
                )
                for w in extra_w:
                    nop = mybir.InstNoOp(
                        name=nc.get_next_instruction_name(), ins=[], outs=[]
                    )
                    nop.engine = inst.engine
                    nop.sync_info = mybir.SyncInfo(on_wait=[w], on_update=[])
                    out.append(nop)
                inst.sync_info = mybir.SyncInfo(on_wait=keep_w, on_update=updates)
                out.append(inst)
            if changed:
                bb.instructions = out


# --- device program -----------------------------------------------------------

def _build(reps=1):
    nc = bass.Bass()
    qT_d = nc.dram_tensor("qT", [DM, NQ], DT, kind="ExternalInput")
    kT_d = nc.dram_tensor("kT", [DM, NK], DT, kind="ExternalInput")
    vT_d = nc.dram_tensor("vT", [DM, NK], DT, kind="ExternalInput")
    w3_d = nc.dram_tensor("w3", [P, NDM * 3 * DK], DT, kind="ExternalInput")
    b3_d = nc.dram_tensor("b3", [DK, 3], F32, kind="ExternalInput")
    out_d = nc.dram_tensor("out", [P, NQT, DK + 1], F32, kind="ExternalOutput")

    EXP = mybir.ActivationFunctionType.Exp
    SCALE = 1.0 / float(np.sqrt(np.float32(DK)))
    QGR = 256            # qT granule width
    VGR = 512            # vT granule width

    with tile.TileContext(nc) as tc:
      for _rep in range(reps):
        with tc.tile_pool(name="persist", bufs=1) as persist:
            w3_sb = persist.tile([P, NDM, 3 * DK], DT, tag="w3_sb")
            b3_sb = persist.tile([DK, 3], F32, tag="b3_sb")
            k_sbT = persist.tile([P, NK], DT, tag="k_sbT")
            q_sbT = persist.tile([P, NQ], DT, tag="q_sbT")
            v_aug = persist.tile([P, NKT, DK + 1], DT, tag="v_aug")
            e_all = persist.tile([P, NKT, NQ], DT, tag="e_all")
            out_sb = persist.tile([P, NQT, DK + 1], F32, tag="out_sb")
            ident = persist.tile([P, P], F32, tag="ident")
            k_nt = persist.tile([P, 4, DK], F32, tag="k_nt")

            with tc.tile_pool(name="psum", bufs=1, space="PSUM") as psp:
                xtp = persist
                # ---- DMA streams -------------------------------------------
                # k granule 0a = key tile 0 alone (2KB, lands ~2.6us on ACT)
                # so scores tile 0 unblocks as soon as q_^T completes.
                # SP:   w3, q0, k0b, k2, v0, v2      (+ out half 0)
                # Pool: q1, q3, k1, k3, v1, v3      (+ out half 1)
                # ACT:  b3, k0a, q2, exp table, then the exp stream.
                kgr = {}
                qgr, vgr = [[None, None] for _ in range(4)], [None] * 4

                def dma_k(nm, lo, hi, eng):
                    t_ = xtp.tile([P, NDM, hi - lo], DT, tag=f"kg{nm}",
                                  name=f"kg{nm}")
                    eng.dma_start(
                        t_[:], kT_d[:, lo:hi].rearrange(
                            "(o p) n -> p o n", p=P))
                    kgr[nm] = t_

                def dma_q(i, h, eng):
                    # dm-half granule: 2KB/partition, lands ~0.9us earlier
                    # than a full-depth granule so the PE starts sooner
                    t_ = xtp.tile([P, NDM // 2, QGR], DT, tag=f"qg{i}{h}",
                                  name=f"qg{i}{h}")
                    eng.dma_start(
                        t_[:], qT_d[h * 512:(h + 1) * 512,
                                    i * QGR:(i + 1) * QGR].rearrange(
                            "(o p) n -> p o n", p=P))
                    qgr[i][h] = t_

                def dma_v(i, eng):
                    t_ = xtp.tile([P, NDM, VGR], DT, tag=f"vg{i}", name=f"vg{i}")
                    eng.dma_start(
                        t_[:], vT_d[:, i * VGR:(i + 1) * VGR].rearrange(
                            "(o p) n -> p o n", p=P))
                    vgr[i] = t_

                nc.sync.dma_start(
                    w3_sb[:], w3_d.rearrange("p (o k) -> p o k", o=NDM))
                nc.scalar.dma_start(b3_sb[:], b3_d[:])
                dma_q(1, 0, nc.gpsimd)
                dma_q(2, 0, nc.scalar)
                dma_q(1, 1, nc.gpsimd)
                dma_q(0, 0, nc.sync)
                dma_q(2, 1, nc.scalar)
                dma_q(3, 0, nc.gpsimd)
                dma_q(0, 1, nc.sync)
                dma_q(3, 1, nc.gpsimd)
                dma_k("0a", 0, 256, nc.scalar)
                dma_k("0b", 256, 512, nc.sync)
                # identity for the k-chunk transposes rides the Pool queue
                # between granules (affine_select is gpsimd-only)
                from concourse.masks import make_identity
                make_identity(nc, ident[:])
                dma_k("1", 512, 1024, nc.gpsimd)
                dma_v(0, nc.sync)
                dma_k("2", 1024, 1536, nc.gpsimd)
                dma_k("3", 1536, 2048, nc.sync)
                dma_v(1, nc.gpsimd)
                dma_v(2, nc.sync)
                dma_v(3, nc.gpsimd)

                # preload the exp table while the DMA streams run
                nc.scalar.activation(
                    e_all[0:1, 0, 0:1], b3_sb[0:1, 0:1], EXP, scale=1.0)
                # ones column -> row 64 of out accumulates the denominator
                nc.vector.memset(v_aug[:, :, DK:DK + 1], 1.0)

                # ---- rotating projection / v-chain PSUM banks --------------
                def pjtile(which, name):
                    return psp.tile([P, 512], F32, tag=f"pj{which}", name=name)

                oacc = [
                    psp.tile([P, 4, DK + 1], F32, tag=f"oacc{g}",
                             name=f"oacc{g}")
                    for g in range(2)
                ]

                def proj(ps, gr, off, wid, wlo, start, stop, glo=0,
                         dmlo=0, nd=NDM):
                    # accumulate granule cols [glo, glo+wid) into psum bank
                    # cols [off, off+wid); gr holds dm tiles [dmlo, dmlo+nd)
                    for dmt in range(nd):
                        nc.tensor.matmul(
                            ps[0:DK, off:off + wid],
                            w3_sb[:, dmlo + dmt, wlo:wlo + DK],
                            gr[:, dmt, glo:glo + wid],
                            start=(start and dmt == 0),
                            stop=(stop and dmt == nd - 1))

                def wb_k(ps, off, wid, dst):
                    nc.vector.tensor_scalar_add(
                        k_sbT[0:DK, dst:dst + wid], ps[0:DK, off:off + wid],
                        b3_sb[:, 1:2])

                def wb_q(ps, off, wid, dst):
                    nc.vector.tensor_scalar_add(
                        q_sbT[0:DK, dst:dst + wid], ps[0:DK, off:off + wid],
                        b3_sb[:, 0:1])

                def v_mm(pv, j, klo, khi, start, stop):
                    # project v chunk-j key tiles [klo,khi) (global indices,
                    # non-transposed); tile kt sits at pv cols (kt%4)*128
                    for dmt in range(NDM):
                        for kt in range(klo, khi):
                            lo = (kt - 4 * j) * P
                            nc.tensor.matmul(
                                pv[:, lo:lo + DK],
                                vgr[j][:, dmt, lo:lo + P],
                                w3_sb[:, dmt, 2 * DK:3 * DK],
                                start=(start and dmt == 0 and kt == klo),
                                stop=(stop and dmt == NDM - 1
                                      and kt == khi - 1))

                def v_wb(pv, j, klo, khi):
                    src = pv[:].rearrange("p (k c) -> p k c", c=P)
                    nc.vector.tensor_copy(
                        v_aug[:, klo:khi, 0:DK],
                        src[:, klo - 4 * j:khi - 4 * j, 0:DK])

                def o_mm(t):
                    # final tile: finish oacc1 first so its writeback + DMA
                    # (the critical tail path) start earliest
                    order = range(NQT - 1, -1, -1) if t == NKT - 1 else \
                        range(NQT)
                    for qi in order:
                        nc.tensor.matmul(
                            oacc[qi // 4][:, qi % 4, :],
                            e_all[:, t, qi * P:(qi + 1) * P],
                            v_aug[:, t, :],
                            start=(t == 0 and qi % 4 == 0),
                            stop=(t == NKT - 1 and qi % 4 == 0))

                def emit_scores(t):
                    sc = psp.tile([P, NQ], F32, tag=f"sc{(t + 1) % 2}",
                                  name=f"sc{t}")
                    # contraction over just the 64 live partitions (d_k)
                    for h in range(2):
                        nc.tensor.matmul(
                            sc[:, h * 512:(h + 1) * 512],
                            k_sbT[0:DK, t * P:(t + 1) * P],
                            q_sbT[0:DK, h * 512:(h + 1) * 512],
                            start=True, stop=True)
                    return sc

                # ---- pre-loop: q projection + k tile 0 ---------------------
                # emission order tracks expected granule arrival; k tiles 0-3
                # accumulate in the first scores buffer (idle until sc1).
                psk0 = psp.tile([P, NQ], F32, tag="sc0", name="psk0")
                psq0 = pjtile(0, "psq0")
                psq1 = pjtile(1, "psq1")
                proj(psq0, qgr[1][0], 256, 256, 0, True, False, nd=4)
                proj(psq1, qgr[2][0], 0, 256, 0, True, False, nd=4)
                proj(psq0, qgr[1][1], 256, 256, 0, False, False, dmlo=4,
                     nd=4)
                proj(psq0, qgr[0][0], 0, 256, 0, False, False, nd=4)
                proj(psq1, qgr[2][1], 0, 256, 0, False, False, dmlo=4, nd=4)
                proj(psq1, qgr[3][0], 256, 256, 0, False, False, nd=4)
                proj(psk0, kgr["0a"], 0, 256, DK, start=True, stop=False)
                wb_k(psk0, 0, 256, 0)
                proj(psq0, qgr[0][1], 0, 256, 0, False, True, dmlo=4, nd=4)
                # q chunk 0 writeback rides ACT (parallel with DVE's chunk 1)
                nc.scalar.activation(
                    q_sbT[0:DK, 0:512], psq0[0:DK, :],
                    mybir.ActivationFunctionType.Identity,
                    bias=b3_sb[:, 0:1])
                proj(psq1, qgr[3][1], 256, 256, 0, False, True, dmlo=4, nd=4)
                wb_q(psq1, 0, 512, 512)

                # ---- main loop: scores -> exp -> (k/v chains, out-mms) -----
                # per-slot extra PE work, keyed by loop slot index
                kwork = {}   # slot -> list of thunks

                def add(slot, fn):
                    kwork.setdefault(slot, []).append(fn)

                def mk(fn):          # bind loop vars eagerly
                    return fn

                # k chunks 1-3: non-transposed projection (64-col matmuls,
                # 2.5x cheaper on the PE) + 4 PE transposes per chunk; the
                # bias folds into the transpose writeback
                psk_t = [None]
                def knt_mm(nm, c):
                    psk_t[0] = pjtile(1, f"pkn{c}")
                    ps = psk_t[0]
                    for dmt in range(NDM):
                        for i in range(4):
                            nc.tensor.matmul(
                                ps[:, i * DK:(i + 1) * DK],
                                kgr[nm][:, dmt, i * P:(i + 1) * P],
                                w3_sb[:, dmt, DK:2 * DK],
                                start=(dmt == 0 and i == 0),
                                stop=(dmt == NDM - 1 and i == 3))
                    nc.vector.tensor_copy(
                        k_nt[:], ps[:, 0:4 * DK].rearrange(
                            "p (k c) -> p k c", c=DK))

                def knt_tr(c):
                    ps = psk_t[0]
                    for i in range(4):
                        nc.tensor.matmul(
                            ps[0:DK, i * P:(i + 1) * P], k_nt[:, i, :],
                            ident[:], is_transpose=True,
                            start=(i == 0), stop=(i == 3))
                    nc.vector.tensor_scalar_add(
                        k_sbT[0:DK, c * 512:(c + 1) * 512], ps[0:DK, :],
                        b3_sb[:, 1:2])

                psv_t = [None]
                def psv_mm(j, klo, khi, start, stop):
                    if start:
                        psv_t[0] = pjtile(0, f"psv{j}_{klo}")
                    pv = psv_t[0]
                    v_mm(pv, j, klo, khi, start, stop)
                    v_wb(pv, j, klo, khi)

                # k chunk c: projection matmuls at slot s0, transposes
                # (which wait on the writeback round-trip) one slot later
                for c, nm, s0 in ((1, "1", 0), (2, "2", 3), (3, "3", 8)):
                    add(s0, mk(lambda nm=nm, c=c: knt_mm(nm, c)))
                    add(s0 + 1, mk(lambda c=c: knt_tr(c)))
                # v sub-chains, two key tiles at a time
                add(2, mk(lambda: psv_mm(0, 0, 2, True, False)))
                add(5, mk(lambda: psv_mm(0, 2, 4, False, True)))
                add(6, mk(lambda: psv_mm(1, 4, 6, True, False)))
                add(7, mk(lambda: psv_mm(1, 6, 8, False, True)))
                add(10, mk(lambda: psv_mm(2, 8, 10, True, False)))
                add(11, mk(lambda: psv_mm(2, 10, 12, False, True)))
                add(13, mk(lambda: psv_mm(3, 12, 14, True, False)))
                add(15, mk(lambda: psv_mm(3, 14, 16, False, True)))

                sc_cur = emit_scores(0)
                # k chunk 0 tiles 2-3 (granule k0b) finish in psk0; this must
                # precede sc1, which rotates back onto psk0's buffer
                proj(psk0, kgr["0b"], 256, 256, DK, start=False, stop=True)
                wb_k(psk0, 256, 256, 256)
                for t in range(NKT + ODELAY):
                    if t < NKT:
                        if t + 1 < NKT:
                            sc_next = emit_scores(t + 1)
                        nc.scalar.activation(
                            e_all[:, t, :], sc_cur[:], EXP, scale=SCALE)
                        if t + 1 < NKT:
                            sc_cur = sc_next
                    for fn in kwork.get(t, []):
                        fn()
                    if t >= ODELAY:
                        o_mm(t - ODELAY)

                # writeback + store (host does the softmax divide); the two
                # halves write back on DVE and ACT in parallel, and the
                # partition-major out layout gives one 1KB descriptor per
                # partition per DMA
                nc.vector.tensor_copy(out_sb[:, 0:4, :], oacc[0][:])
                nc.scalar.copy(out_sb[:, 4:8, :], oacc[1][:])
                nc.sync.dma_start(out_d[:, 0:4, :], out_sb[:, 0:4, :])
                nc.scalar.dma_start(out_d[:, 4:8, :], out_sb[:, 4:8, :])
    _legalize_waits(nc)
    return nc


_nc_cache = None


def _get_nc():
    global _nc_cache
    if _nc_cache is None:
        _nc_cache = _build()
    return _nc_cache


def _marshal(q, k, v, Wq, bq, Wk, bk, Wv, bv):
    """Host-side layout prep: transpose to [B, d_model, N], cast to bf16,
    shard over (batch, query-half)."""
    qT = np.ascontiguousarray(np.transpose(np.asarray(q), (0, 2, 1))).astype(BF)
    kT = np.ascontiguousarray(np.transpose(np.asarray(k), (0, 2, 1))).astype(BF)
    vT = np.ascontiguousarray(np.transpose(np.asarray(v), (0, 2, 1))).astype(BF)
    w3 = np.concatenate(
        [np.asarray(Wq), np.asarray(Wk), np.asarray(Wv)], axis=1
    ).astype(BF)
    # [1024, 192] -> [128, 8*192] partition-major so the DMA is contiguous
    w3 = np.ascontiguousarray(
        w3.reshape(NDM, P, 3 * DK).transpose(1, 0, 2).reshape(P, NDM * 3 * DK)
    )
    b3 = np.stack(
        [np.asarray(bq), np.asarray(bk), np.asarray(bv)], axis=1
    ).astype(np.float32)
    in_maps = []
    for c in range(NCORES):
        bi, h = divmod(c, 2)
        in_maps.append({
            "qT": np.ascontiguousarray(qT[bi][:, h * NQ:(h + 1) * NQ]),
            "kT": kT[bi],
            "vT": vT[bi],
            "w3": w3, "b3": b3,
        })
    return in_maps


def _unmarshal(results, bv):
    out = np.empty((B, N, DK), np.float32)
    for c in range(NCORES):
        bi, h = divmod(c, 2)
        aug = np.transpose(np.asarray(results[c]['out'], dtype=np.float32), (1, 0, 2)).reshape(NQ, DK + 1)
        out[bi, h * NQ:(h + 1) * NQ] = (
            aug[:, :DK] / aug[:, DK:DK + 1] + np.asarray(bv)[None, :]
        )
    return out


def kernel(q, k, v, Wq, bq, Wk, bk, Wv, bv):
    in_maps = _marshal(q, k, v, Wq, bq, Wk, bk, Wv, bv)
    res = run_bass_kernel_spmd(_get_nc(), in_maps, core_ids=list(range(NCORES)))
    return _unmarshal(res.results, bv)
